# revision 1
# baseline (speedup 1.0000x reference)
"""Trainium2 Bass kernel for nn_Block_61881888801303 (dense_cnn).

The reference is a chain of five 1x1 convs (C=3) with residual adds on a
(16, 3, 1024, 1024) input. Every produced tensor is affine in x, so the
four distinct output planes collapse to a single per-pixel affine map

    y[o] = W12[o, :] @ x[:, pixel] + v12[o]      o = 0..11

with W12 = rows [a(3); bb(3); c2(3); out4(3)] composed on the host from
the 3x3 conv weights. The reference's res list aliases c2 three times, so
the device computes only [a, bb, c2, out4] and the host duplicates c2.

Device mapping (per core, batch-sharded 2 images/core):
  - pixels of one image are split into G=8 groups of 131072; a matmul with
    block-diagonal stationary weights lhsT[24, 96] (K = 8 groups x 3
    channels, M = 8 groups x 12 outputs) processes 8 pixels/cycle on the
    PE: rhs[24, 512] -> psum[96, 512].
  - ACT/DVE copy psum -> SBUF adding the per-partition bias v12.
  - One 786KB input DMA and one 3MB output DMA per 16-matmul super-tile.
  - input tiles live at partitions 64..87 (odd DMA ports) to balance the
    output traffic on partitions 0..95 (2/3 even ports).
"""

import numpy as np

import concourse.bass as bass
import concourse.mybir as mybir
from concourse.tile import TileContext
from concourse.bass_utils import run_bass_kernel_spmd

# ---------------------------------------------------------------- constants
N_CORES = 8
B, C, H, W = 16, 3, 1024, 1024
B_LOC = B // N_CORES          # 2 images per core
G = 8                         # pixel groups (block-diagonal replicas)
NOUT = 12                     # 4 outputs x 3 channels
K = C * G                     # 24 contraction rows
M = NOUT * G                  # 96 psum partitions
NFREE = 512                   # matmul free dim (one PSUM bank, fp32)
S = 16                        # matmuls per super-tile
WTILE = NFREE * S             # 8192 elements per row per super-tile
NSUP = (H * W) // (G * WTILE) # 16 super-tiles per image
IN_P0 = 64                    # input tile base partition (odd DMA ports)

FP32 = mybir.dt.float32


def _compose(w1, b1, w2, b2, w3, b3, w4, b4, w5, b5):
    """Fold the conv chain into one [12,3] matrix + [12] bias (fp64 math)."""
    f = np.float64
    W1, W2, W3, W4, W5 = (np.asarray(m, f) for m in (w1, w2, w3, w4, w5))
    c1, c2, c3, c4, c5 = (np.asarray(v, f) for v in (b1, b2, b3, b4, b5))
    Ma, va = W1, c1
    Mb, vb = W2 @ Ma, W2 @ va + c2
    Mc0, vc0 = W3 @ Mb + Ma, W3 @ vb + c3 + va
    Mo3, vo3 = W4 @ Mc0, W4 @ vc0 + c4
    Mo4, vo4 = W5 @ Mo3, W5 @ vo3 + c5
    Mc2, vc2 = 2 * Mc0 + 3 * Ma + 3 * Mb, 2 * vc0 + 3 * va + 3 * vb
    W12 = np.concatenate([Ma, Mb, Mc2, Mo4], 0).astype(np.float32)
    v12 = np.concatenate([va, vb, vc2, vo4], 0).astype(np.float32)
    return W12, v12


def _build_wmat_bias(W12, v12):
    """lhsT [K=24, M=96]: block-diagonal over G pixel groups; bias [96,1]."""
    wmat = np.zeros((K, M), np.float32)
    for g in range(G):
        for c in range(C):
            for o in range(NOUT):
                wmat[c * G + g, o * G + g] = W12[o, c]
    bvec = np.zeros((M, 1), np.float32)
    for o in range(NOUT):
        for g in range(G):
            bvec[o * G + g, 0] = v12[o]
    return wmat, bvec


def _split_multi_sem_waits(nc):
    """walrus in this toolchain rejects >1 sem wait per instruction; hoist
    extra waits onto same-engine NOPs inserted immediately before."""
    import bass_rust

    for f in nc.m.functions:
        for bb in f.blocks:
            il = bb.instructions
            i = 0
            while i < len(il):
                inst = il[i]
                si = inst.sync_info
                if si is not None and si.on_wait and len(si.on_wait) > 1:
                    waits = list(si.on_wait)
                    si.on_wait = [waits[-1]]
                    inst.sync_info = si
                    for k, w in enumerate(waits[:-1]):
                        nop = mybir.InstNoOp(
                            name=f"{inst.name}_wsplit{k}",
                            engine=inst.engine,
                            sync_info=bass_rust.SyncInfo(
                                on_wait=[w], on_update=[]
                            ),
                        )
                        il.insert(i, nop)
                        i += 1
                i += 1


def _build_nc():
    nc = bass.Bass()
    x = nc.declare_dram_parameter(
        "xs", [B_LOC, C, G, NSUP, WTILE], FP32, isOutput=False
    )
    wm = nc.declare_dram_parameter("wmat", [K, M], FP32, isOutput=False)
    bv = nc.declare_dram_parameter("bvec", [M, 1], FP32, isOutput=False)
    y = nc.declare_dram_parameter(
        "y", [B_LOC, NOUT, G, NSUP, WTILE], FP32, isOutput=True
    )

    with TileContext(nc) as tc:
        with (
            tc.tile_pool(name="const", bufs=1) as const_pool,
            tc.tile_pool(name="inb", bufs=3) as in_pool,
            tc.tile_pool(name="outb", bufs=2) as out_pool,
            tc.tile_pool(name="ps", bufs=8, space=bass.MemorySpace.PSUM) as ps_pool,
        ):
            w_full = const_pool.tile([128, M], FP32)
            bias_t = const_pool.tile([M, 1], FP32)
            nc.sync.dma_start(w_full[IN_P0 : IN_P0 + K, :], wm[:])
            nc.sync.dma_start(bias_t[:], bv[:])
            w_ap = w_full[IN_P0 : IN_P0 + K, :]

            for b in range(B_LOC):
                for s in range(NSUP):
                    in_t = in_pool.tile([128, WTILE], FP32)
                    nc.sync.dma_start(
                        in_t[IN_P0 : IN_P0 + K, :],
                        x[b, :, :, s, :].rearrange("c g w -> (c g) w"),
                    )
                    out_t = out_pool.tile([M, WTILE], FP32)
                    for j in range(S):
                        ps = ps_pool.tile([M, NFREE], FP32)
                        nc.tensor.matmul(
                            ps[:],
                            w_ap,
                            in_t[IN_P0 : IN_P0 + K, j * NFREE : (j + 1) * NFREE],
                            start=True,
                            stop=True,
                        )
                        dst = out_t[:, j * NFREE : (j + 1) * NFREE]
                        if j % 2 == 0:
                            nc.scalar.activation(
                                dst,
                                ps[:],
                                mybir.ActivationFunctionType.Identity,
                                bias=bias_t[:],
                                scale=1.0,
                            )
                        else:
                            nc.vector.tensor_scalar_add(dst, ps[:], bias_t[:])
                    nc.sync.dma_start(
                        y[b, :, :, s, :].rearrange("o g w -> (o g) w"),
                        out_t[:],
                    )

    _split_multi_sem_waits(nc)
    return nc


def kernel(x, w1, b1, w2, b2, w3, b3, w4, b4, w5, b5):
    x = np.ascontiguousarray(np.asarray(x, np.float32))
    W12, v12 = _compose(w1, b1, w2, b2, w3, b3, w4, b4, w5, b5)
    wmat, bvec = _build_wmat_bias(W12, v12)

    nc = _build_nc()
    in_maps = []
    for i in range(N_CORES):
        xs = x[i * B_LOC : (i + 1) * B_LOC].reshape(B_LOC, C, G, NSUP, WTILE)
        in_maps.append({"xs": xs, "wmat": wmat, "bvec": bvec})

    res = run_bass_kernel_spmd(nc, in_maps, list(range(N_CORES)))

    out4 = np.empty((B, C, H, W), np.float32)
    resv = np.empty((5, B, C, H, W), np.float32)
    for i in range(N_CORES):
        yc = res.results[i]["y"]  # [B_LOC, 12, G, NSUP, WTILE]
        yc = yc.reshape(B_LOC, 4, C, H, W)
        sl = slice(i * B_LOC, (i + 1) * B_LOC)
        resv[0, sl] = yc[:, 0]              # a
        resv[1, sl] = yc[:, 1]              # bb
        c2 = yc[:, 2]
        resv[2, sl] = c2
        resv[3, sl] = c2
        resv[4, sl] = c2
        out4[sl] = yc[:, 3]
    return out4, resv


# revision 4
# speedup vs baseline: 10.8012x; 10.8012x over previous
"""Trainium2 Bass kernel for nn_Block_61881888801303 (dense_cnn).

The reference is a chain of five 1x1 convs (C=3) with residual adds on a
(16, 3, 1024, 1024) input. Every produced tensor is affine in x, so the
four distinct output planes collapse to a single per-pixel affine map

    y[o] = W12[o, :] @ x[:, pixel] + v12[o]      o = 0..11

with W12 = rows [a(3); bb(3); c2(3); out4(3)] composed on the host from
the 3x3 conv weights. The reference's res list aliases c2 three times, so
the device computes only [a, bb, c2, out4] and the host duplicates c2.

Device mapping (per core, batch-sharded 2 images/core), memory-bound at
~126MB of HBM traffic (~352us at 358GB/s). fp32 matmuls cost 4 cyc/row on
the PE, so the PE alone cannot cover all pixels inside the DMA window;
pixels are split between two fp32-exact paths:

  PE path (first K_PE/16 of each image): block-diagonal stationary
    weights lhsT[24, 96] (8 pixel groups x 3 ch -> 8 x 12 outputs);
    each matmul rhs[24, 512] -> psum[96, 512] covers 4096 pixels.
    ACT/DVE evict psum to SBUF adding the per-partition bias.
  Direct path (rest): per output plane, chained fused affine ops
    ACT (m0*A0 + v), DVE stt (+ m1*A1), POOL stt (+ m2*A2) on
    [128, 512] pixel tiles.

  Input tiles for the PE path live at partitions 64..87 (odd DMA ports)
  to balance output traffic on partitions 0..95 (2/3 even ports).
"""

import numpy as np

import concourse.bass as bass
import concourse.mybir as mybir
from concourse.tile import TileContext
from concourse.bass_utils import run_bass_kernel_spmd

# ---------------------------------------------------------------- constants
N_CORES = 8
B, C, H, W = 16, 3, 1024, 1024
B_LOC = B // N_CORES          # 2 images per core
PIX = H * W                   # 1048576 pixels per image
G = 8                         # pixel groups (block-diagonal replicas)
NOUT = 12                     # 4 outputs x 3 channels
K = C * G                     # 24 contraction rows
M = NOUT * G                  # 96 psum partitions
NFREE = 512                   # matmul free dim (one PSUM bank, fp32)
S = 16                        # matmuls per PE super-tile
WTILE = NFREE * S             # 8192 elements per group-row per super-tile
NSUP = PIX // (G * WTILE)     # 16 super-tile-equivalents per image
K_PE = 11                     # super-tiles per image on the PE path
NDIR = NSUP - K_PE            # direct-path units per image (65536 px each)
PE_PIX = K_PE * G * WTILE     # pixels per image on the PE path
GROW = K_PE * WTILE           # group-row length in the PE region
DTU = 512                     # direct-path tile free size
IN_P0 = 64                    # PE input tile base partition (odd DMA ports)

FP32 = mybir.dt.float32


def _compose(w1, b1, w2, b2, w3, b3, w4, b4, w5, b5):
    """Fold the conv chain into one [12,3] matrix + [12] bias (fp64 math)."""
    f = np.float64
    W1, W2, W3, W4, W5 = (np.asarray(m, f) for m in (w1, w2, w3, w4, w5))
    c1, c2, c3, c4, c5 = (np.asarray(v, f) for v in (b1, b2, b3, b4, b5))
    Ma, va = W1, c1
    Mb, vb = W2 @ Ma, W2 @ va + c2
    Mc0, vc0 = W3 @ Mb + Ma, W3 @ vb + c3 + va
    Mo3, vo3 = W4 @ Mc0, W4 @ vc0 + c4
    Mo4, vo4 = W5 @ Mo3, W5 @ vo3 + c5
    Mc2, vc2 = 2 * Mc0 + 3 * Ma + 3 * Mb, 2 * vc0 + 3 * va + 3 * vb
    W12 = np.concatenate([Ma, Mb, Mc2, Mo4], 0).astype(np.float32)
    v12 = np.concatenate([va, vb, vc2, vo4], 0).astype(np.float32)
    return W12, v12


def _build_wmat_bias(W12, v12):
    """lhsT [K=24, M=96] block-diagonal over G pixel groups; psum bias
    [96,1]; direct-path bias [128, 12] (v12 replicated per partition)."""
    wmat = np.zeros((K, M), np.float32)
    for g in range(G):
        for c in range(C):
            for o in range(NOUT):
                wmat[c * G + g, o * G + g] = W12[o, c]
    bvec = np.zeros((M, 1), np.float32)
    for o in range(NOUT):
        for g in range(G):
            bvec[o * G + g, 0] = v12[o]
    vdir = np.tile(v12[None, :], (128, 1)).astype(np.float32)
    return wmat, bvec, vdir


def _split_multi_sem_waits(nc):
    """walrus in this toolchain rejects >1 sem wait per instruction; hoist
    extra waits onto same-engine NOPs inserted immediately before."""
    import bass_rust

    for f in nc.m.functions:
        for bb in f.blocks:
            il = bb.instructions
            i = 0
            while i < len(il):
                inst = il[i]
                si = inst.sync_info
                if si is not None and si.on_wait and len(si.on_wait) > 1:
                    waits = list(si.on_wait)
                    si.on_wait = [waits[-1]]
                    inst.sync_info = si
                    for k, w in enumerate(waits[:-1]):
                        nop = mybir.InstNoOp(
                            name=f"{inst.name}_wsplit{k}",
                            engine=inst.engine,
                            sync_info=bass_rust.SyncInfo(
                                on_wait=[w], on_update=[]
                            ),
                        )
                        il.insert(i, nop)
                        i += 1
                i += 1


def _build_nc(W12f=None):
    """W12f: [12,3] float weights for the direct path's immediates."""
    if W12f is None:
        W12f = np.zeros((NOUT, C), np.float32)
    nc = bass.Bass()
    x = nc.declare_dram_parameter("xs", [B_LOC, C, PIX], FP32, isOutput=False)
    wm = nc.declare_dram_parameter("wmat", [K, M], FP32, isOutput=False)
    bv = nc.declare_dram_parameter("bvec", [M, 1], FP32, isOutput=False)
    vd = nc.declare_dram_parameter("vdir", [128, NOUT], FP32, isOutput=False)
    y = nc.declare_dram_parameter("y", [B_LOC, NOUT, PIX], FP32, isOutput=True)

    act = mybir.ActivationFunctionType.Identity
    add = mybir.AluOpType.add
    mult = mybir.AluOpType.mult

    with TileContext(nc) as tc:
        with (
            tc.tile_pool(name="const", bufs=1) as const_pool,
            tc.tile_pool(name="inb", bufs=2) as in_pool,
            tc.tile_pool(name="outb", bufs=2) as out_pool,
            tc.tile_pool(name="din", bufs=3) as din_pool,
            tc.tile_pool(name="dout", bufs=2) as dout_pool,
            tc.tile_pool(name="ps", bufs=8, space=bass.MemorySpace.PSUM) as ps_pool,
        ):
            w_full = const_pool.tile([128, M], FP32)
            bias_t = const_pool.tile([M, 1], FP32)
            vdir_t = const_pool.tile([128, NOUT], FP32)
            nc.sync.dma_start(w_full[IN_P0 : IN_P0 + K, :], wm[:])
            nc.sync.dma_start(bias_t[:], bv[:])
            nc.sync.dma_start(vdir_t[:], vd[:])
            w_ap = w_full[IN_P0 : IN_P0 + K, :]

            for b in range(B_LOC):
                xpe = x[b, :, 0:PE_PIX].rearrange(
                    "c (g s w) -> c g s w", g=G, s=K_PE, w=WTILE
                )
                ype = y[b, :, 0:PE_PIX].rearrange(
                    "o (g s w) -> o g s w", g=G, s=K_PE, w=WTILE
                )
                for s in range(NSUP):
                    if s < K_PE:
                        # ---------------- PE path super-tile ----------------
                        in_t = in_pool.tile([128, WTILE], FP32)
                        nc.sync.dma_start(
                            in_t[IN_P0 : IN_P0 + K, :],
                            xpe[:, :, s, :],
                        )
                        out_t = out_pool.tile([M, WTILE], FP32)
                        for j in range(S):
                            ps = ps_pool.tile([M, NFREE], FP32)
                            nc.tensor.matmul(
                                ps[:],
                                w_ap,
                                in_t[
                                    IN_P0 : IN_P0 + K,
                                    j * NFREE : (j + 1) * NFREE,
                                ],
                                start=True,
                                stop=True,
                            )
                            dst = out_t[:, j * NFREE : (j + 1) * NFREE]
                            if j % 16 < 10:
                                nc.scalar.activation(
                                    dst, ps[:], act, bias=bias_t[:], scale=1.0
                                )
                            else:
                                nc.vector.tensor_scalar_add(dst, ps[:], bias_t[:])
                        nc.sync.dma_start(
                            ype[:, :, s, :],
                            out_t[:],
                        )
                    else:
                        # ---------------- direct path unit ------------------
                        lo = PE_PIX + (s - K_PE) * 128 * DTU
                        hi = lo + 128 * DTU
                        din_t = din_pool.tile([128, C * DTU], FP32)
                        nc.sync.dma_start(
                            din_t[:],
                            x[b, :, lo:hi].rearrange(
                                "c (p w) -> p c w", p=128, w=DTU
                            ),
                        )
                        dout_t = dout_pool.tile([128, NOUT * DTU], FP32)
                        A = [din_t[:, c * DTU : (c + 1) * DTU] for c in range(C)]
                        for o in range(NOUT):
                            O = dout_t[:, o * DTU : (o + 1) * DTU]
                            nc.scalar.activation(
                                O,
                                A[0],
                                act,
                                bias=vdir_t[:, o : o + 1],
                                scale=float(W12f[o, 0]),
                            )
                            nc.vector.scalar_tensor_tensor(
                                O, A[1], float(W12f[o, 1]), O, mult, add
                            )
                            nc.vector.scalar_tensor_tensor(
                                O, A[2], float(W12f[o, 2]), O, mult, add
                            )
                        nc.sync.dma_start(
                            y[b, :, lo:hi].rearrange(
                                "o (p w) -> p o w", p=128, w=DTU
                            ),
                            dout_t[:],
                        )

    _split_multi_sem_waits(nc)
    return nc


def kernel(x, w1, b1, w2, b2, w3, b3, w4, b4, w5, b5):
    x = np.ascontiguousarray(np.asarray(x, np.float32))
    W12, v12 = _compose(w1, b1, w2, b2, w3, b3, w4, b4, w5, b5)
    wmat, bvec, vdir = _build_wmat_bias(W12, v12)

    nc = _build_nc(W12)
    in_maps = []
    for i in range(N_CORES):
        xs = x[i * B_LOC : (i + 1) * B_LOC].reshape(B_LOC, C, PIX)
        in_maps.append({"xs": xs, "wmat": wmat, "bvec": bvec, "vdir": vdir})

    res = run_bass_kernel_spmd(nc, in_maps, list(range(N_CORES)))

    out4 = np.empty((B, C, H, W), np.float32)
    resv = np.empty((5, B, C, H, W), np.float32)
    for i in range(N_CORES):
        yc = res.results[i]["y"]  # [B_LOC, 12, PIX]
        yc = yc.reshape(B_LOC, 4, C, H, W)
        sl = slice(i * B_LOC, (i + 1) * B_LOC)
        resv[0, sl] = yc[:, 0]              # a
        resv[1, sl] = yc[:, 1]              # bb
        c2 = yc[:, 2]
        resv[2, sl] = c2
        resv[3, sl] = c2
        resv[4, sl] = c2
        out4[sl] = yc[:, 3]
    return out4, resv
